# revision 67
# baseline (speedup 1.0000x reference)
"""Causal depthwise conv1d (K=4) + SiLU, sharded over 8 NeuronCores.

Full shapes: x [4, 8192, 2048] f32, weight [2048, 4] f32 -> y [4, 8192, 2048] f32.

Strategy: tensor-parallel over the hidden/channel dim (fully channel
independent, no halo exchange). Each core gets 256 channels -> 1024
independent rows (batch x channel). All HBM traffic is bf16 (the 2e-2
rel-err budget dwarfs bf16's ~1e-3), halving the memory-bound roofline
vs f32.

Layout: time is phase-split host-side, t = 4j + p. SBUF partition dim
packs (32 rows x 4 phases); the free dim is the block index j. A causal
conv tap then only ever reads the current block j or block j-1, so each
512-block PSUM chunk needs just TWO 128x128 banded-matmul accumulations
(prev-block taps + cur-block taps) instead of one diag matmul per tap:
2x less TensorEngine time than the diagonal formulation, keeping PE
(~55us) under the bf16 DMA roofline (~94us). The banded weight matrices
are block-diagonal over rows with 4x4 tap bands over phases.

Compute: PE accumulates bf16 matmuls into f32 PSUM (4 rotating 1024-col
buffers across all 8 banks); ACT applies SiLU straight out of PSUM,
writing bf16, in 1024-col chunks, and triggers each store one step late
so the ~900ns semaphore propagation never blocks the next activation.
x and y are tile-major within each DRAM partition row ([128, NU*J]) so
bulk loads and stores are two-tile transfers with 8KB contiguous lines
(quarter the DMA descriptor/completion overhead of per-tile 4KB lines);
the last tiles load/store fine-grained so the tail pipeline drains
early, with each concurrent transfer class on its own semaphore (the
halves complete out of order). 12 y slots keep the activation engine's
slot-reuse gate ~8us ahead of the measured ~4.6us store-queue latency.
The causal zero block is handled by skipping the prev-block matmul's
first output column in chunk 0. The stationary matrices are assembled
on-device (GpSimd affine_select diagonals + DVE scatter-accumulate from
a 29KB scalar table) to keep 512KB of weight DMA off the stream.

Measured: 95.1us best (198.5us baseline, 2.09x); late-session samples
swing 95-112us with neighbor HBM contention on the shared chip. The
kernel is at the chip-HBM roofline — ~8.7us engine/NEFF preamble +
~34MB/core of bf16 traffic at the 8-core-shared HBM rate + a short
drain. fp8 I/O would halve bytes again but fails the 2e-2 gate (~6%
error at the distribution's max elements). Tested and reverted: 16KB
quad input transfers (coarser input semaphores stall PE more than the
descriptor savings return) and an earlier weight-build/ACT start
(early stores weave into the DMA engine queues and delay input-stream
completion, which the tail chain hangs off).

Raw bass (no Tile framework): the installed walrus codegen only accepts one
sync wait per compute instruction, so all synchronization is explicit wait_ge
sequencer instructions. Per-buffer-slot DMA semaphores keep concurrent DMA
completion increments unambiguous. Sem increments fire at instruction
completion, but the sequencer runs ahead, so consumers of an engine's result
always gate on that completion increment (including same-engine self-waits
before DMA triggers).
"""

import contextlib

import numpy as np
import ml_dtypes

B, S, H, K = 4, 8192, 2048, 4
N_CORES = 8
HC = H // N_CORES          # 256 channels per core
ROWS = B * HC              # 1024 rows per core, row r = b*HC + c
P = 4                      # time phases per partition group, t = P*j + p
J = S // P                 # 2048 blocks
RPU = 128 // P             # 32 rows per partition unit
NU = ROWS // RPU           # 32 units (tiles); tile k = unit k, all blocks
NG = HC // RPU             # 8 distinct weight groups (weights repeat per b)
NB = 8                     # x buffers (slots)
NBY = 12                   # y buffers: extra slack so ACT never waits on
                           # the store queue (latency ~4.6us vs 2-tile gate)
NC_CHUNK = 512             # one PSUM bank of fp32
NCHUNKS = J // NC_CHUNK    # 4
PC = 1024                  # psum buffer / activation chunk (2 banks)
NH = J // PC               # 2 chunks per tile
NPS = 4                    # psum buffers (all 8 banks); ping depth 4

BF16 = ml_dtypes.bfloat16

_last_results = None       # test harness introspection (exec_time_ns etc.)
_ACT_FUNC = "Silu"         # sim override hook (CoreSim lacks Silu)


def _build_program():
    from concourse import bass, mybir

    f32 = mybir.dt.float32
    bf16 = mybir.dt.bfloat16
    AF = mybir.ActivationFunctionType

    nc = bass.Bass()
    # phase-split x, tile-major within each partition row: row q = 4*rho + p
    # holds x[32u+rho, P*j+p] at col u*J + j. Tile-pair loads are then single
    # transfers with 8KB contiguous lines (half the descriptors of per-tile
    # 4KB-line loads).
    x_d = nc.declare_dram_parameter("x", [128, NU * J], bf16, isOutput=False)
    # compact per-diagonal scalars (cur: NG*K cols, prev: NG*(K-1) cols,
    # last col zeros for the Silu bias); the dense banded stationaries are
    # assembled on-device by the otherwise-idle GpSimd+DVE to keep 512KB of
    # weight-table DMA off the HBM-saturated stream
    SCC = NG * K + NG * (K - 1) + 1
    sc_d = nc.declare_dram_parameter("sc", [128, SCC], f32, isOutput=False)
    # y uses the same tile-major row layout as x so bulk stores can be
    # two-tile transfers with 8KB contiguous lines
    y_d = nc.declare_dram_parameter("y", [128, NU * J], bf16, isOutput=True)

    with contextlib.ExitStack() as st:
        wsb = st.enter_context(nc.sbuf_tensor("wsb", [128, NG * 2 * 128], bf16))
        scsb = st.enter_context(nc.sbuf_tensor("scsb", [128, SCC], f32))
        ones = st.enter_context(nc.sbuf_tensor("ones", [128, 128], bf16))
        msk = st.enter_context(nc.sbuf_tensor("msk", [128, 7 * 128], bf16))
        tmp = st.enter_context(nc.sbuf_tensor("tmp", [128, 128], bf16))
        # contiguous x/y buffers so adjacent slot pairs can be the target
        # or source of a single two-tile DMA
        xbig = st.enter_context(nc.sbuf_tensor("xb", [128, NB * J], bf16))
        ybig = st.enter_context(nc.sbuf_tensor("yb", [128, NBY * J], bf16))
        pss = [
            st.enter_context(nc.psum_tensor(f"ps{i}", [128, PC], f32))
            for i in range(NPS)
        ]

        def wmat(g, which):               # which: 0=prev-block, 1=cur-block
            c0 = (g * 2 + which) * 128
            return wsb[:, c0 : c0 + 128]

        def mdiag(d):                     # shifted-diag mask, delta = d - 3
            return msk[:, d * 128 : (d + 1) * 128]

        def xslot(s):
            return xbig[:, s * J : (s + 1) * J]

        with (
            # gpsimd only runs the early mask build (no DMA): skip its
            # expensive end-of-block dge_drain
            nc.Block(no_gpsimd_drain=True) as block,
            nc.semaphore("wsem") as wsem,
            nc.semaphore("act") as act,
            nc.semaphore("pe") as pe,
            nc.semaphore("dl") as dl,
            nc.semaphore("esem") as esem,
            nc.semaphore("dve") as dve,
            contextlib.ExitStack() as sems,
        ):
            # one input semaphore per slot PAIR (two-tile transfers), plus
            # dedicated sems for the fine-grained tail loads (tile NU-2 and
            # the halves of tile NU-1): concurrent transfers must never mix
            # increments on a semaphore a consumer thresholds mid-way.
            din = [
                sems.enter_context(nc.semaphore(f"din{i}"))
                for i in range(NB // 2)
            ]
            d30 = sems.enter_context(nc.semaphore("d30"))
            dout = [
                sems.enter_context(nc.semaphore(f"dout{i}"))
                for i in range(NBY // 2)
            ]

            @block.gpsimd
            def _(gpsimd):
                # shifted-diagonal masks for the on-device weight build
                # (affine_select only exists on gpsimd). NOTE: starting the
                # build (and thus ACT and the first stores) EARLIER measurably
                # hurt end-to-end time — early stores weave into the DMA
                # engine queues and delay the input stream's completion,
                # which the tail chain hangs off. Keep the lazy order.
                gpsimd.memset(ones[:, :], 1.0)
                for d in range(7):
                    delta = d - 3
                    gpsimd.affine_select(
                        out=mdiag(d), in_=ones[:, :],
                        pattern=[[1, 128]], base=-delta, channel_multiplier=-1,
                        compare_op=mybir.AluOpType.is_equal, fill=0.0,
                    ).then_inc(esem)

            @block.vector
            def _(vector):
                # assemble the 16 banded stationaries: each is a sum of
                # masked shifted diagonals scaled by a per-partition column
                vector.wait_ge(wsem, 16)
                vector.wait_ge(esem, 7)

                def build_wc(g):
                    wc = wmat(g, 1)
                    # cur-block: delta = 0..3, tap K-1-delta
                    vector.tensor_scalar_mul(
                        wc, mdiag(3), scsb[:, g * K : g * K + 1]
                    )
                    for delta in range(1, K):
                        vector.tensor_scalar_mul(
                            tmp[:, :], mdiag(3 + delta),
                            scsb[:, g * K + delta : g * K + delta + 1],
                        )
                        vector.tensor_add(wc, wc, tmp[:, :])

                def build_wp(g):
                    wp = wmat(g, 0)
                    # prev-block: delta = -1..-3, tap -delta-1
                    c0 = NG * K + g * (K - 1)
                    vector.tensor_scalar_mul(
                        wp, mdiag(2), scsb[:, c0 : c0 + 1]
                    )
                    for dp in range(2, K):
                        vector.tensor_scalar_mul(
                            tmp[:, :], mdiag(3 - dp),
                            scsb[:, c0 + dp - 1 : c0 + dp],
                        )
                        mm = vector.tensor_add(wp, wp, tmp[:, :])
                    return mm

                for g in range(NG):
                    build_wc(g)
                    build_wp(g).then_inc(dve)

            @block.sync
            def _(sync):
                sync.dma_start(out=scsb[:, :], in_=sc_d[:, :]).then_inc(wsem, 16)
                # tiles 0..NU-3 load as two-tile transfers (8KB lines); the
                # final two tiles load fine-grained so the tail pipeline
                # starts before the whole 1MB pair lands
                for m in range(NU // 2 - 1):
                    s = (2 * m) % NB
                    if m >= NB // 2:
                        # slot pair free once PE consumed both prior tiles
                        sync.wait_ge(pe, NH * (2 * m - NB + 2))
                    sync.dma_start(
                        out=xbig[:, s * J : (s + 2) * J],
                        in_=x_d[:, (2 * m) * J : (2 * m + 2) * J],
                    ).then_inc(din[m % (NB // 2)], 16)
                # tiles NU-2 (slot NB-2) and NU-1 (slot NB-1): pair last used
                # by tiles NU-10/NU-9
                sync.wait_ge(pe, NH * (NU - NB))
                sync.dma_start(
                    out=xslot(NB - 2),
                    in_=x_d[:, (NU - 2) * J : (NU - 1) * J],
                ).then_inc(d30, 16)
                for h, sem in ((0, din[NB // 2 - 1]), (1, dl)):
                    sync.dma_start(
                        out=xbig[:, (NB - 1) * J + h * PC : (NB - 1) * J + (h + 1) * PC],
                        in_=x_d[:, (NU - 1) * J + h * PC : (NU - 1) * J + (h + 1) * PC],
                    ).then_inc(sem, 16)

            @block.tensor
            def _(tensor):
                # pe/act semaphores count PC-col chunks, NH per tile; psum
                # buffers rotate over NPS chunks
                for k in range(NU):
                    if k < NG:
                        # stationary pair g=k built by DVE
                        tensor.wait_ge(dve, k + 1)
                    if k < NU - 2:
                        m = k // 2
                        tensor.wait_ge(
                            din[m % (NB // 2)], 16 * (m // (NB // 2) + 1)
                        )
                    elif k == NU - 2:
                        tensor.wait_ge(d30, 16)
                    xt = xslot(k % NB)
                    g = k % NG
                    for h in range(NH):
                        G = k * NH + h
                        if k == NU - 1:
                            # split load: chunk 0 needs half A, chunk 1 both
                            if h == 0:
                                tensor.wait_ge(din[NB // 2 - 1], 64)
                            else:
                                tensor.wait_ge(dl, 16)
                        if G >= NPS:
                            # psum buffer free once silu of chunk G-NPS done
                            tensor.wait_ge(act, G - NPS + 1)
                        ps = pss[G % NPS]
                        for c2 in range(PC // NC_CHUNK):
                            c0 = h * PC + c2 * NC_CHUNK   # within the tile
                            p0 = c2 * NC_CHUNK            # within the psum buf
                            if c0 == 0:
                                # block -1 is the causal zero block: psum col
                                # 0 gets no prev contribution. cur starts the
                                # group (zeroes the whole 512-col bank).
                                mm = tensor.matmul(
                                    ps[:, 0:NC_CHUNK],
                                    wmat(g, 1),
                                    xt[:, 0:NC_CHUNK],
                                    start=True,
                                    stop=False,
                                    skip_group_check=True,
                                )
                                mm = tensor.matmul(
                                    ps[:, 1:NC_CHUNK],
                                    wmat(g, 0),
                                    xt[:, 0 : NC_CHUNK - 1],
                                    start=False,
                                    stop=True,
                                    skip_group_check=True,
                                )
                            else:
                                mm = tensor.matmul(
                                    ps[:, p0 : p0 + NC_CHUNK],
                                    wmat(g, 0),
                                    xt[:, c0 - 1 : c0 - 1 + NC_CHUNK],
                                    start=True,
                                    stop=False,
                                    skip_group_check=True,
                                )
                                mm = tensor.matmul(
                                    ps[:, p0 : p0 + NC_CHUNK],
                                    wmat(g, 1),
                                    xt[:, c0 : c0 + NC_CHUNK],
                                    start=False,
                                    stop=True,
                                    skip_group_check=True,
                                )
                        mm.then_inc(pe)

            @block.scalar
            def _(scalar):
                func = getattr(AF, _ACT_FUNC)

                # hybrid stores: two-tile 8KB-line transfers for the bulk
                # (quarter the descriptor/completion overhead of 2KB chunk
                # stores), chunk-granular only for the last TAILK tiles where
                # store latency paces the drain. All triggers run while a
                # later activation occupies the engine, so the waited-on
                # completion incs have already propagated.
                TAILK = NU - 4

                def store_pair(m):                # tiles 2m, 2m+1
                    s = (2 * m) % NBY
                    scalar.wait_ge(act, NH * (2 * m + 2))
                    scalar.dma_start(
                        out=y_d[:, (2 * m) * J : (2 * m + 2) * J],
                        in_=ybig[:, s * J : (s + 2) * J],
                    ).then_inc(dout[m % (NBY // 2)], 16)

                def store_chunk(G):
                    k, h = G // NH, G % NH
                    scalar.wait_ge(act, G + 1)
                    scalar.dma_start(
                        out=y_d[:, k * J + h * PC : k * J + (h + 1) * PC],
                        in_=ybig[:, (k % NBY) * J + h * PC : (k % NBY) * J + (h + 1) * PC],
                    ).then_inc(dout[(k % NBY) // 2], 16)

                for k in range(NU):
                    for h in range(NH):
                        G = k * NH + h
                        scalar.wait_ge(pe, G + 1)
                        if h == 0 and k >= NBY:
                            # y slot-pair's previous store must be done
                            scalar.wait_ge(
                                dout[(k % NBY) // 2], 16 * (k // NBY)
                            )
                        scalar.activation(
                            out=ybig[:, (k % NBY) * J + h * PC : (k % NBY) * J + (h + 1) * PC],
                            in_=pss[G % NPS][:, :],
                            func=func,
                            bias=0.0 if func == AF.Copy else scsb[:, SCC - 1 : SCC],
                            scale=1.0,
                        ).then_inc(act)
                        if h == 0 and k >= 2 and k % 2 == 0 and k - 2 < TAILK:
                            store_pair((k - 2) // 2)
                        if G >= 1 and (G - 1) // NH >= TAILK:
                            store_chunk(G - 1)
                store_chunk(NU * NH - 1)
                for sp in range(NBY // 2):
                    n = 16 * len(
                        [m for m in range(TAILK // 2) if m % (NBY // 2) == sp]
                    ) + 16 * NH * len(
                        [k for k in range(TAILK, NU) if (k % NBY) // 2 == sp]
                    )
                    scalar.wait_ge(dout[sp], n)

    return nc


def _scalar_table(ws):
    """ws: (HC, K) f32 -> [128, NG*(2K-1)+1] f32 per-diagonal scalar columns.

    Partition index q = 4*rho + p_in; the device scatters column s onto the
    shifted diagonal [q, q+delta]. Cur-block (delta = p_out - p_in in 0..K-1)
    carries tap i = K-1-delta, valid while (q%P)+delta <= P-1; prev-block
    (delta = -1..-(K-1)) carries tap i = -delta-1, valid while (q%P) >= -delta.
    The final column is zeros (Silu bias operand).
    """
    q = np.arange(128)
    sc = np.zeros((128, NG * (2 * K - 1) + 1), np.float32)
    for g in range(NG):
        ch = ws[RPU * g : RPU * (g + 1)]          # (RPU, K)
        wq = ch[q // P, :]                        # (128, K) per-partition taps
        for delta in range(K):
            sc[:, g * K + delta] = wq[:, K - 1 - delta] * ((q % P) + delta <= P - 1)
        for dp in range(1, K):
            sc[:, NG * K + g * (K - 1) + dp - 1] = wq[:, dp - 1] * ((q % P) >= dp)
    return sc


def kernel(x, weight):
    global _last_results
    from concourse.bass_utils import run_bass_kernel_spmd

    x = np.asarray(x, dtype=np.float32)
    weight = np.asarray(weight, dtype=np.float32)

    nc = _build_program()

    in_maps = []
    for core in range(N_CORES):
        sl = slice(core * HC, (core + 1) * HC)
        # [B, S, HC] -> [B, HC, S] -> [ROWS, S], row r = b*HC + c
        xc = x[:, :, sl].transpose(0, 2, 1).reshape(ROWS, S)
        # phase split: partition row 4r+p, block col j = x[r, 4j+p], then
        # tile-major: device row q, col u*J+j holds unit u's partition q
        xs = np.ascontiguousarray(
            xc.reshape(ROWS, J, P).transpose(0, 2, 1).reshape(NU, 128, J)
            .transpose(1, 0, 2).reshape(128, NU * J)
        ).astype(BF16)
        in_maps.append({"x": xs, "sc": _scalar_table(weight[sl, :])})

    res = run_bass_kernel_spmd(nc, in_maps, list(range(N_CORES)))
    _last_results = res

    out = np.empty((B, S, H), np.float32)
    for core in range(N_CORES):
        sl = slice(core * HC, (core + 1) * HC)
        yc = np.asarray(res.results[core]["y"], dtype=np.float32)
        # undo tile-major row layout, then phase split, then [B, HC, S] ->
        # [B, S, HC]
        yc = yc.reshape(128, NU, J).transpose(1, 0, 2).reshape(ROWS * P, J)
        yc = yc.reshape(ROWS, P, J).transpose(0, 2, 1).reshape(B, HC, S)
        out[:, :, sl] = yc.transpose(0, 2, 1)
    return out
